# revision 25
# baseline (speedup 1.0000x reference)
"""Trainium2 Bass kernel for a 2-layer relational GNN (EvalNet).

Strategy: shard by destination node (core c owns nodes [2048c, 2048c+2048)).
All edges with dst in a core's range are processed on that core, so layer-1
and layer-2 aggregations are core-local (no all-reduce). The only collective
is a bf16 AllGather of x1. Segment-sums are done as one-hot matmuls
accumulating in PSUM; per-edge source rows are fetched with dma_gather
spread over 4 SWDGE queues. One-hot masks are precomputed on the host and
DMA'd from HBM (cheap) instead of built per-chunk on the vector engine
(expensive); x[dst] rows are selected from the resident destination tile
with a mask matmul instead of a second gather.
"""

import os
import sys

sys.path.insert(0, "/opt/trn_rl_repo")

import numpy as np
import ml_dtypes

import concourse.bacc as bacc
import concourse.tile as tile
import concourse.mybir as mybir
from concourse import bass_utils
from concourse.library_config import mlp as mlp_lib

BF16 = ml_dtypes.bfloat16

N = 16384
E = 262144
R = 9
DIN = 384
H = 768
NCLS = 5
NCORES = 8
NOWN = N // NCORES          # 2048 nodes per core
NT = NOWN // 128            # 16 dst tiles of 128 per core

FP32 = mybir.dt.float32
BF = mybir.dt.bfloat16
I16 = mybir.dt.int16
F8 = mybir.dt.float8e4
F8NP = ml_dtypes.float8_e4m3fn
X1SCL = 0.125   # x1 stored in fp8 as x1/8 to stay well inside e4m3 range
AX = mybir.AxisListType
ALU = mybir.AluOpType
ACTF = mybir.ActivationFunctionType

NSQ = int(os.environ.get("NSQ", 4))   # SWDGE queues


def _wrap16(ids):
    """int16 index layout for dma_gather: [128, n/16], element i at
    [i%16 (+16r replicas), i//16]."""
    a = np.asarray(ids, np.int16).reshape(-1, 16).T  # [16, n/16]
    return np.ascontiguousarray(np.tile(a, (8, 1)))


def _roundup(x, m):
    return (x + m - 1) // m * m


def _prep(src, dst, et, nag=2):
    """Build the uniform schedule + per-core index/mask arrays.

    nag: number of partial AllGathers; x1 lands in the gathered buffer in
    slab-major/rank-major order, so layer-2 gather indices are remapped:
    node n = c*NOWN + g*rows + l  ->  g*(NCORES*rows) + c*rows + l.
    """
    rows = NOWN // nag

    def remap(n):
        n = np.asarray(n)
        c, loc = n // NOWN, n % NOWN
        g, l = loc // rows, loc % rows
        return g * (NCORES * rows) + c * rows + l

    core = dst // NOWN
    tloc = (dst % NOWN) // 128
    slot = dst % 128

    per_core_edges = [np.nonzero(core == c)[0] for c in range(NCORES)]
    counts1 = np.zeros((NCORES, NT, R), np.int64)
    for c in range(NCORES):
        ecs = per_core_edges[c]
        np.add.at(counts1[c], (tloc[ecs], et[ecs]), 1)
    K1 = _roundup(counts1.max(axis=0), 128)  # [NT, R] slots per group
    S1 = K1.sum(axis=1)                      # [NT] slots per tile
    E1 = int(S1.sum())
    NCH1 = E1 // 128

    sched1 = []
    for t in range(NT):
        gs, c0 = [], 0
        for r in range(R):
            nch = int(K1[t, r]) // 128
            if nch:
                gs.append((r, c0, c0 + nch))
                c0 += nch
        sched1.append(gs)

    counts2 = np.zeros((NCORES, NT), np.int64)
    for c in range(NCORES):
        ecs = per_core_edges[c]
        np.add.at(counts2[c], tloc[ecs], 1)
    K2 = _roundup(counts2.max(axis=0), 128)  # [NT]
    E2 = int(K2.sum())
    NCH2 = E2 // 128

    # L2 chunk -> max source slab (over cores), for prefix-AP dependencies:
    # a chunk whose sources all lie in slabs <= g only needs AllGathers 0..g.
    slab = (src % NOWN) // rows               # [E] source slab id
    gmax2 = np.zeros(NCH2, np.int64)

    cores = []
    for c in range(NCORES):
        ecs = per_core_edges[c]
        tl, sl, rr = tloc[ecs], slot[ecs], et[ecs]

        src1 = np.zeros(E1, np.int64)
        slot1 = np.full(E1, -1, np.int64)   # dst slot in tile (-1 = pad)
        rel1 = np.full(E1, -1, np.int64)
        src2 = np.zeros(E2, np.int64)
        slot2 = np.full(E2, -1, np.int64)

        off1 = 0
        for t in range(NT):
            for r in range(R):
                k = int(K1[t, r])
                if k == 0:
                    continue
                sel = (tl == t) & (rr == r)
                es = ecs[sel]
                n = es.size
                src1[off1:off1 + n] = src[es]
                slot1[off1:off1 + n] = sl[sel]
                rel1[off1:off1 + n] = r
                off1 += k
        assert off1 == E1

        off2 = 0
        for t in range(NT):
            k = int(K2[t])
            sel = tl == t
            es = ecs[sel]
            # sort this tile's edges by source slab so early chunks only
            # depend on early partial AllGathers
            order = np.argsort(slab[es], kind="stable")
            es = es[order]
            n = es.size
            src2[off2:off2 + n] = src[es]
            slot2[off2:off2 + n] = sl[sel][order]
            sbp = np.zeros(k, np.int64)
            sbp[:n] = slab[es]
            ch0 = off2 // 128
            np.maximum.at(gmax2, ch0 + np.arange(k) // 128, sbp)
            off2 += k
        assert off2 == E2

        # one-hot masks, host-built: [e, s] per chunk, bf16
        s1c = slot1.reshape(NCH1, 128)
        m1 = (s1c[:, :, None] == np.arange(128)[None, None, :])
        l1mask = np.ascontiguousarray(
            m1.transpose(1, 0, 2).reshape(128, NCH1 * 128)).astype(BF16)
        # transposed variant for the dst-row select matmul: [s, e] per chunk
        l1maskT = np.ascontiguousarray(
            m1.transpose(2, 0, 1).reshape(128, NCH1 * 128)).astype(BF16)
        r1c = rel1.reshape(NCH1, 128)
        mr = (r1c[:, :, None] == np.arange(R)[None, None, :])
        l1rel = np.ascontiguousarray(
            mr.transpose(1, 0, 2).reshape(128, NCH1 * R)).astype(BF16)
        s2c = slot2.reshape(NCH2, 128)
        m2 = (s2c[:, :, None] == np.arange(128)[None, None, :])
        l2mask = np.ascontiguousarray(
            m2.transpose(1, 0, 2).reshape(128, NCH2 * 128)).astype(F8NP)

        deg = np.zeros(NOWN, np.float32)
        np.add.at(deg, dst[ecs] - c * NOWN, 1.0)
        degones = np.stack([deg, np.ones(NOWN, np.float32)])  # [2, NOWN]

        cores.append(dict(
            src1=_wrap16(src1), l1mask=l1mask, l1maskT=l1maskT, l1rel=l1rel,
            src2=_wrap16(remap(src2)), l2mask=l2mask,
            degones=degones,
            ownidx=_wrap16(np.arange(NOWN)),   # local rows of cc_in
        ))

    return dict(E1=E1, NCH1=NCH1, E2=E2, NCH2=NCH2, S1=S1, K2=K2,
                sched1=sched1, cores=cores, nag=nag, gmax2=gmax2)


def _build(sch, phase="full"):
    E1, NCH1, E2, NCH2 = sch["E1"], sch["NCH1"], sch["E2"], sch["NCH2"]
    S1, K2, sched1 = sch["S1"], sch["K2"], sch["sched1"]
    nag = sch.get("nag", 2)
    gmax2 = sch["gmax2"]
    agrows = NOWN // nag * NCORES   # cc_out rows written per partial AG
    G1 = max(int(s) for s in S1) // 128   # max chunks per L1 tile
    G2 = max(int(k) for k in K2) // 128   # max chunks per L2 tile

    nc = bacc.Bacc("TRN2", target_bir_lowering=False, debug=False,
                   num_devices=NCORES, num_swdge_queues=NSQ,
                   dynamic_dma_scratch_size=int(
                       os.environ.get("DMA_SCRATCH", 16384)))

    xb_d = nc.dram_tensor("xb", [N, DIN], BF, kind="ExternalInput")
    xown_d = nc.dram_tensor("xown", [NOWN, DIN], BF, kind="ExternalInput")
    relwt_d = nc.dram_tensor("relwt", [R, DIN, H], BF, kind="ExternalInput")
    relb_d = nc.dram_tensor("relb", [R, H], FP32, kind="ExternalInput")
    lint_d = nc.dram_tensor("lint", [H, H], BF, kind="ExternalInput")
    selft_d = nc.dram_tensor("selft", [H, H], BF, kind="ExternalInput")
    b2_d = nc.dram_tensor("b2", [2, H], FP32, kind="ExternalInput")
    degones_d = nc.dram_tensor("degones", [2, NOWN], FP32, kind="ExternalInput")
    src1_d = nc.dram_tensor("src1", [128, E1 // 16], I16, kind="ExternalInput")
    l1mask_d = nc.dram_tensor("l1mask", [128, NCH1 * 128], BF, kind="ExternalInput")
    l1maskT_d = nc.dram_tensor("l1maskT", [128, NCH1 * 128], BF, kind="ExternalInput")
    l1rel_d = nc.dram_tensor("l1rel", [128, NCH1 * R], BF, kind="ExternalInput")
    src2_d = nc.dram_tensor("src2", [128, E2 // 16], I16, kind="ExternalInput")
    l2mask_d = nc.dram_tensor("l2mask", [128, NCH2 * 128], F8, kind="ExternalInput")
    ownidx_d = nc.dram_tensor("ownidx", [128, NOWN // 16], I16, kind="ExternalInput")
    pooled_d = nc.dram_tensor("pooled", [128, 6], FP32, kind="ExternalOutput")
    x1dump_d = None
    if phase == "l1":
        x1dump_d = nc.dram_tensor("x1dump", [NOWN, H], BF, kind="ExternalOutput")
    elif phase == "ag":
        x1dump_d = nc.dram_tensor("x1dump", [N, H], BF, kind="ExternalOutput")

    qctr = [0]

    def nextq():
        q = qctr[0] % NSQ
        qctr[0] += 1
        return q

    with tile.TileContext(nc) as tc:
        nc.gpsimd.load_library(mlp_lib)
        with (
            tc.tile_pool(name="const", bufs=1) as cp,
            tc.tile_pool(name="dram", bufs=1, space="DRAM") as dp,
        ):
            # index/meta arrays first — they gate the first gathers
            src1_sb = cp.tile([128, E1 // 16], I16)
            nc.sync.dma_start(src1_sb[:], src1_d[:])
            src2_sb = cp.tile([128, E2 // 16], I16)
            nc.sync.dma_start(src2_sb[:], src2_d[:])
            ownidx_sb = cp.tile([128, NOWN // 16], I16)
            nc.sync.dma_start(ownidx_sb[:], ownidx_d[:])
            relwt_sb = cp.tile([128, R * 3 * H], BF)
            nc.sync.dma_start(
                relwt_sb.rearrange("p (r k h) -> p r k h", k=3, h=H)[:, :, :, :],
                relwt_d.rearrange("r (k p) h -> p r k h", p=128)[:, :, :, :])
            lint_sb = cp.tile([128, 6 * H], BF)
            selft_sb = cp.tile([128, 6 * H], BF)
            nc.sync.dma_start(
                lint_sb.rearrange("p (k h) -> p k h", h=H)[:, :, :],
                lint_d.rearrange("(k p) h -> p k h", p=128)[:, :, :])
            nc.sync.dma_start(
                selft_sb.rearrange("p (k h) -> p k h", h=H)[:, :, :],
                selft_d.rearrange("(k p) h -> p k h", p=128)[:, :, :])
            relb_sb = cp.tile([R, H], FP32)
            nc.sync.dma_start(relb_sb[:], relb_d[:])
            b2_sb = cp.tile([2, H], FP32)
            nc.sync.dma_start(b2_sb[:], b2_d[:])
            degones_sb = cp.tile([2, NOWN], FP32)
            nc.sync.dma_start(degones_sb[:], degones_d[:])
            pooled_sb = cp.tile([128, 6], FP32)
            nc.vector.memset(pooled_sb[:], 0.0)

            cc_in = dp.tile([NOWN, H], F8)
            cc_inb = dp.tile([NOWN, H], BF)
            cc_out = dp.tile([N, H], F8)

            # ================= Layer 1 =================
            with (
                tc.tile_pool(name="w1", bufs=2) as wp,
                tc.tile_pool(name="ps1", bufs=2, space="PSUM") as pp,
            ):
                gblk = int(os.environ.get("GBLK", 6))

                def sub_gather(dst_tile, src_ap, idx_sb, chunk0, nchunks, elem):
                    # split gathers to fit the SWDGE descriptor ring; rotate
                    # queues so desc-gen overlaps across rings
                    v3 = dst_tile.rearrange("p (c d) -> p c d", d=elem)
                    for b0 in range(0, nchunks, gblk):
                        b1 = min(b0 + gblk, nchunks)
                        col = (chunk0 + b0) * 8
                        nc.gpsimd.dma_gather(
                            v3[:, b0:b1, :], src_ap,
                            idx_sb[:, col:col + (b1 - b0) * 8],
                            (b1 - b0) * 128, (b1 - b0) * 128, elem,
                            queue_num=nextq())

                knt = int(os.environ.get("KNT", NT))
                LA = int(os.environ.get("LA", 0))   # gather lookahead (tiles)
                cb = [0]
                for t in range(NT):
                    cb.append(cb[t] + int(S1[t]) // 128)

                def emit_fetch(t):
                    """Issue tile t's gathers + mask loads (runs LA tiles
                    ahead of compute so the AllGather instructions between
                    tile groups don't stall the gather stream)."""
                    chunk_base, ncht = cb[t], int(S1[t]) // 128
                    xs_g = wp.tile([128, G1 * DIN], BF, tag="xs", bufs=max(2, LA + 1))
                    sub_gather(xs_g, xb_d[:], src1_sb, chunk_base, ncht, DIN)
                    # resident dst tile rows (contiguous, no gather)
                    xdt = wp.tile([128, DIN], BF, tag="xdt", bufs=max(2, LA + 1))
                    nc.sync.dma_start(xdt[:], xown_d[t * 128:(t + 1) * 128, :])
                    # per-tile masks
                    m1_sb = wp.tile([128, G1 * 128], BF, tag="m1", bufs=max(2, LA + 1))
                    nc.sync.dma_start(
                        m1_sb[:, :ncht * 128],
                        l1mask_d[:, chunk_base * 128:(chunk_base + ncht) * 128])
                    m1T_sb = wp.tile([128, G1 * 128], BF, tag="m1T", bufs=max(2, LA + 1))
                    nc.sync.dma_start(
                        m1T_sb[:, :ncht * 128],
                        l1maskT_d[:, chunk_base * 128:(chunk_base + ncht) * 128])
                    r1_sb = wp.tile([128, G1 * R], BF, tag="r1", bufs=max(2, LA + 1))
                    nc.sync.dma_start(
                        r1_sb[:, :ncht * R],
                        l1rel_d[:, chunk_base * R:(chunk_base + ncht) * R])
                    return xs_g, xdt, m1_sb, m1T_sb, r1_sb

                fetched = {}

                def emit_compute(t):
                    xs_g, xdt, m1_sb, m1T_sb, r1_sb = fetched.pop(t)
                    ncht = int(S1[t]) // 128
                    art_sb = wp.tile([128, R * 3 * 128], BF, tag="artsb")
                    ct_ps = pp.tile([R, 128], FP32, tag="ct")
                    nct = ncht  # chunks in tile

                    for (r, gc0, gc1) in sched1[t]:
                        art_ps = pp.tile([128, 3 * 128], FP32, tag="art", bufs=2)
                        for ci in range(gc0, gc1):
                            xs_c = xs_g[:, ci * DIN:(ci + 1) * DIN]
                            # select x[dst] rows for this chunk via mask matmul
                            selp = pp.tile([128, 512], FP32, tag="selp", bufs=2)
                            nc.tensor.matmul(
                                selp[:, :DIN],
                                m1T_sb[:, ci * 128:(ci + 1) * 128],
                                xdt[:], start=True, stop=True)
                            # ew = <x_src, x_dst> per edge
                            prod = wp.tile([128, DIN], BF, tag="prod", bufs=4)
                            nc.vector.tensor_tensor(
                                prod[:], xs_c, selp[:, :DIN], ALU.mult)
                            ew = wp.tile([128, 1], FP32, tag="ew", bufs=4)
                            nc.vector.reduce_sum(ew[:], prod[:], axis=AX.X)
                            # weighted one-hot: oh[e, s] = mask[e, s] * ew[e]
                            oh_c = wp.tile([128, 128], BF, tag="ohc", bufs=4)
                            nc.scalar.activation(
                                oh_c[:], m1_sb[:, ci * 128:(ci + 1) * 128],
                                ACTF.Copy, scale=ew[:])
                            nc.tensor.matmul(ct_ps[:],
                                             r1_sb[:, ci * R:(ci + 1) * R],
                                             oh_c[:],
                                             start=(ci == 0),
                                             stop=(ci == nct - 1))
                            for k in range(3):
                                nc.tensor.matmul(
                                    art_ps[:, k * 128:(k + 1) * 128],
                                    xs_c[:, k * 128:(k + 1) * 128],
                                    oh_c[:], start=(ci == gc0 and k == 0),
                                    stop=(ci == gc1 - 1 and k == 2))
                        nc.scalar.copy(
                            art_sb[:, r * 384:(r + 1) * 384], art_ps[:])

                    ct_sb = wp.tile([R, 128], FP32, tag="ctsb")
                    nc.vector.tensor_copy(ct_sb[:], ct_ps[:])

                    x1t = wp.tile([128, H], BF, tag="x1t")
                    x1t8 = wp.tile([128, H], F8, tag="x1t8")
                    for s in range(2):
                        mps = pp.tile([128, 384], FP32, tag="mps")
                        first = True
                        for (r, _, _) in sched1[t]:
                            for k in range(3):
                                nc.tensor.matmul(
                                    mps[:],
                                    art_sb[:, r * 384 + k * 128:r * 384 + (k + 1) * 128],
                                    relwt_sb[:, (r * 3 + k) * H + s * 384:
                                             (r * 3 + k) * H + (s + 1) * 384],
                                    start=first, stop=False)
                                first = False
                        nc.tensor.matmul(mps[:], ct_sb[:],
                                         relb_sb[:, s * 384:(s + 1) * 384],
                                         start=False, stop=True)
                        nc.scalar.activation(x1t[:, s * 384:(s + 1) * 384],
                                             mps[:], ACTF.Relu)
                        nc.scalar.activation(x1t8[:, s * 384:(s + 1) * 384],
                                             mps[:], ACTF.Relu, scale=X1SCL)
                    nc.sync.dma_start(cc_inb[t * 128:(t + 1) * 128, :], x1t[:])
                    nc.sync.dma_start(cc_in[t * 128:(t + 1) * 128, :], x1t8[:])

                # software-pipelined emission: fetch runs LA tiles ahead, and
                # each partial AllGather is emitted after the lookahead
                # fetches so its semaphore wait doesn't head-of-line-block
                # the gather stream on the in-order gpsimd queue.
                tper = NT // nag
                AGD = int(os.environ.get("AGD", 1))  # AG emission delay, tiles

                def emit_ag(g):
                    # slab g lands at cc_out rows [g*8*rows, (g+1)*8*rows)
                    # in rank-major order (gather indices are remapped).
                    rows = NOWN // nag
                    nc.gpsimd.collective_compute(
                        "AllGather", ALU.bypass,
                        replica_groups=[list(range(NCORES))],
                        ins=[cc_in[g * rows:(g + 1) * rows, :].opt()],
                        outs=[cc_out[g * NCORES * rows:
                                     (g + 1) * NCORES * rows, :].opt()])

                # Each partial AllGather is emitted AGD tiles after its slab
                # completes: by then its semaphore wait is already satisfied,
                # so it doesn't head-of-line-block the gather stream on the
                # in-order gpsimd queue.
                next_ag = 0
                for t in range(knt + LA):
                    if t < knt:
                        fetched[t] = emit_fetch(t)
                    tcp = t - LA
                    if tcp < 0:
                        continue
                    emit_compute(tcp)
                    if phase in ("ag", "full"):
                        while (next_ag < nag
                               and tcp + 1 >= (next_ag + 1) * tper + AGD):
                            emit_ag(next_ag)
                            next_ag += 1
                last_in_l2 = phase == "full"
                if phase in ("ag", "full"):
                    while next_ag < (nag - 1 if last_in_l2 else nag):
                        emit_ag(next_ag)
                        next_ag += 1

            # ================= AllGather x1 =================
            if phase == "l1":
                nc.sync.dma_start(x1dump_d[:], cc_inb[:])
            if phase == "ag" and x1dump_d is not None:
                nc.sync.dma_start(x1dump_d[:], cc_out[:])
            x1src = cc_out if phase in ("ag", "full") else cc_in

            # ================= Layer 2 =================
            with (
                tc.tile_pool(name="w2", bufs=2) as wp2,
                tc.tile_pool(name="ps2", bufs=2, space="PSUM") as pp2,
            ):
                gblk2 = int(os.environ.get("GBLK2", 6))
                PRE = int(os.environ.get("PRE", 4))
                cb2 = [0]
                for t in range(NT):
                    cb2.append(cb2[t] + int(K2[t]) // 128)

                def emit_l2_gather(v3, t, b0, b1, gm):
                    col = (cb2[t] + b0) * 8
                    # only depend on the AllGather slabs this group actually
                    # reads (sources are slab-sorted)
                    pref = (x1src[0:(gm + 1) * agrows, :]
                            if phase in ("ag", "full") else x1src[:])
                    nc.gpsimd.dma_gather(
                        v3[:, b0:b1, :], pref,
                        src2_sb[:, col:col + (b1 - b0) * 8],
                        (b1 - b0) * 128, (b1 - b0) * 128, H,
                        queue_num=nextq())

                fetched2 = {}
                pend_b = []

                def l2_fetch(t, defer_last_slab):
                    ncht = int(K2[t]) // 128
                    x1s_g = wp2.tile([128, G2 * H], F8, tag="x1s", bufs=int(os.environ.get("X1B", 4)))
                    v3 = x1s_g.rearrange("p (c d) -> p c d", d=H)
                    for b0 in range(0, ncht, gblk2):
                        b1 = min(b0 + gblk2, ncht)
                        gm = int(gmax2[cb2[t] + b0:cb2[t] + b1].max())
                        if defer_last_slab and gm >= nag - 1:
                            pend_b.append((v3, t, b0, b1, gm))
                            continue
                        emit_l2_gather(v3, t, b0, b1, gm)
                    m2_sb = wp2.tile([128, G2 * 128], F8, tag="m2", bufs=int(os.environ.get("X1B", 4)))
                    nc.sync.dma_start(
                        m2_sb[:, :ncht * 128],
                        l2mask_d[:, cb2[t] * 128:(cb2[t] + ncht) * 128])
                    fetched2[t] = (x1s_g, m2_sb)

                if phase == "full":
                    # start the first tiles' early-slab gathers BEFORE the
                    # last AllGather is emitted, so they aren't serialized
                    # behind it on the in-order gpsimd queue
                    for t in range(PRE):
                        l2_fetch(t, defer_last_slab=True)
                    emit_ag(nag - 1)
                    for args in pend_b:
                        emit_l2_gather(*args)
                    pend_b = []

                for w in (range(4) if phase in ("full", "nocc") else []):
                    bt_sb = wp2.tile([128, 6 * 512], BF, tag="btsb")
                    for tt in range(4):
                        t = w * 4 + tt
                        ncht = int(K2[t]) // 128
                        if t not in fetched2:
                            l2_fetch(t, defer_last_slab=False)
                        x1s_g, m2_sb = fetched2.pop(t)
                        bta = pp2.tile([128, 512], FP32, tag="bta")
                        btb = pp2.tile([128, 256], FP32, tag="btb")
                        for ci in range(ncht):
                            x1s_c = x1s_g[:, ci * H:(ci + 1) * H]
                            oh_c = m2_sb[:, ci * 128:(ci + 1) * 128]
                            for j in range(6):
                                tgt = (bta[:, j * 128:(j + 1) * 128] if j < 4
                                       else btb[:, (j - 4) * 128:(j - 3) * 128])
                                nc.tensor.matmul(
                                    tgt, x1s_c[:, j * 128:(j + 1) * 128], oh_c,
                                    start=(ci == 0 and j in (0, 4)),
                                    stop=(ci == ncht - 1 and j in (3, 5)))
                        for j in range(6):
                            src_ps = (bta[:, j * 128:(j + 1) * 128] if j < 4
                                      else btb[:, (j - 4) * 128:(j - 3) * 128])
                            nc.scalar.mul(
                                bt_sb[:, j * 512 + tt * 128:j * 512 + (tt + 1) * 128],
                                src_ps, 1.0 / X1SCL)

                    # self-transform input: own x1 rows, transposed — read
                    # from the local slab (no AllGather dependency)
                    x1tw = wp2.tile([128, 6 * 512], BF, tag="x1tw")
                    nc.gpsimd.dma_gather(
                        x1tw.rearrange("p (c i) -> p c i", i=512)[:, :, :],
                        cc_inb[:], ownidx_sb[:, w * 32:(w + 1) * 32],
                        512, 512, H, transpose=True, queue_num=nextq())

                    for j in range(6):
                        aps = pp2.tile([128, 512], FP32, tag="agg2")
                        first = True
                        for k in range(6):
                            nc.tensor.matmul(
                                aps[:],
                                lint_sb[:, k * H + j * 128:k * H + (j + 1) * 128],
                                bt_sb[:, k * 512:(k + 1) * 512],
                                start=first, stop=False)
                            first = False
                            nc.tensor.matmul(
                                aps[:],
                                selft_sb[:, k * H + j * 128:k * H + (j + 1) * 128],
                                x1tw[:, k * 512:(k + 1) * 512],
                                start=False, stop=False)
                        nc.tensor.matmul(
                            aps[:], b2_sb[:, j * 128:(j + 1) * 128],
                            degones_sb[:, w * 512:(w + 1) * 512],
                            start=False, stop=True)
                        x2 = wp2.tile([128, 512], FP32, tag="x2")
                        nc.scalar.activation(x2[:], aps[:], ACTF.Relu)
                        red = wp2.tile([128, 1], FP32, tag="red")
                        nc.vector.reduce_sum(red[:], x2[:], axis=AX.X)
                        nc.vector.tensor_add(pooled_sb[:, j:j + 1],
                                             pooled_sb[:, j:j + 1], red[:])

            nc.sync.dma_start(pooled_d[:], pooled_sb[:])

    nc.compile()
    return nc


def make_in_maps(inputs, sch):
    nrm = np.asarray(inputs["norm_constants"], np.float32)
    relw = np.asarray(inputs["rel_W"], np.float32) / nrm[:, None, None]
    relb = np.asarray(inputs["rel_b"], np.float32) / nrm[:, None]
    relwt = np.ascontiguousarray(relw.transpose(0, 2, 1)).astype(BF16)
    lint = np.ascontiguousarray(np.asarray(inputs["mp_lin_W"], np.float32).T).astype(BF16)
    selft = np.ascontiguousarray(np.asarray(inputs["mp_self_W"], np.float32).T).astype(BF16)
    b2 = np.stack([np.asarray(inputs["mp_lin_b"], np.float32),
                   np.asarray(inputs["mp_self_b"], np.float32)])
    xbm = np.asarray(inputs["x"], np.float32).astype(BF16)
    in_maps = []
    for c in range(NCORES):
        cd = sch["cores"][c]
        in_maps.append(dict(
            xb=xbm, xown=xbm[c * NOWN:(c + 1) * NOWN],
            relwt=relwt, relb=relb,
            lint=lint, selft=selft, b2=b2, degones=cd["degones"],
            src1=cd["src1"], l1mask=cd["l1mask"], l1maskT=cd["l1maskT"],
            l1rel=cd["l1rel"], src2=cd["src2"], l2mask=cd["l2mask"],
            ownidx=cd["ownidx"]))
    return in_maps


def prep_from_inputs(inputs):
    ei = np.asarray(inputs["edge_index"], np.int64)
    et = np.asarray(inputs["edge_type"], np.int64)
    return _prep(ei[0], ei[1], et, nag=int(os.environ.get("NAG", 4)))


def kernel(**inputs) -> np.ndarray:
    out_W = np.asarray(inputs["out_W"], np.float32)
    out_b = np.asarray(inputs["out_b"], np.float32)

    sch = prep_from_inputs(inputs)
    phase = os.environ.get("KPHASE", "full")
    import time as _t
    t0 = _t.time()
    nc = _build(sch, phase=phase)
    print(f"[kernel] build+compile {_t.time()-t0:.1f}s", flush=True)

    in_maps = make_in_maps(inputs, sch)

    t0 = _t.time()
    res = bass_utils.run_bass_kernel_spmd(
        nc, in_maps, core_ids=list(range(NCORES)),
        tmpdir=os.environ.get("BASS_TMPDIR"))
    print(f"[kernel] run {_t.time()-t0:.1f}s", flush=True)

    pooled = np.zeros(H, np.float64)
    for c in range(NCORES):
        p = res.results[c]["pooled"]  # [128, 6]
        pooled += p.T.reshape(-1).astype(np.float64)  # h = j*128 + p

    kernel._x1dump = [res.results[c].get("x1dump") for c in range(NCORES)]
    kernel._last_exec_ns = res.exec_time_ns

    out = (pooled / N).astype(np.float32) @ out_W.T + out_b
    return out.astype(np.float32)
